# revision 12
# baseline (speedup 1.0000x reference)
"""Trainium2 Bass kernel for nn_InstanceDeformableConvBlock.

Sharding: 8 cores = 4 batches x 2 image halves (32 output rows each).
Per core: conv1 (3x3, bf16, 9 shifted matmuls, zero-padded 66-wide layout),
bn1+relu -> out1; offset/mod conv (quarter-packed 27-row psum); per-pixel
fp32 planes (magic-number floor, clamp-into-zero-pad border); ap_gather
(d=2 channel pairs; y1 neighbor = +66 input offset, x1 = idx+1); gamma via
DRAM-wrap + doubling-DMA replication; deform einsum = 9 accumulated bf16
matmuls; 1x1 downsample+bn3; final relu(bn2(deform)+res).
"""
import sys
sys.path.insert(0, '/opt/trn_rl_repo')
import numpy as np
import ml_dtypes

BF16 = ml_dtypes.bfloat16

B, CIN, COUT, H, W = 4, 256, 256, 64, 64
NROW = 40           # out1 rows: idx i -> global row 32h-3+i
NXR = 42            # x_pad rows: idx i -> global row 32h-4+i
WPAD = 66
NPIX1 = NROW * WPAD
NP = 2048
MAGIC = float(1.5 * 2.0 ** 23)

_NC = None


def _build():
    from concourse import bacc, mybir, tile
    dt = mybir.dt
    Alu = mybir.AluOpType
    Act = mybir.ActivationFunctionType

    nc = bacc.Bacc("TRN2", target_bir_lowering=False, debug=False, num_devices=8)

    def din(name, shape, dtype):
        return nc.dram_tensor(name, shape, dtype, kind="ExternalInput")

    x_pad_d = din("x_pad", [128, 2 * NXR * WPAD], dt.bfloat16)
    w1_d = din("w1", [128, 2 * 9 * 2 * 128], dt.bfloat16)
    om_d = din("om", [128, 2 * 9 * 73], dt.bfloat16)
    dw_d = din("dw", [128, 2 * 9 * 2 * 128], dt.bfloat16)
    dn_d = din("dn", [128, 2 * 2 * 128], dt.bfloat16)
    bnv_d = din("bnv", [128, 14], dt.float32)
    vec27_d = din("vec27", [96, 4], dt.float32)
    basey_d = din("basey", [9, NP], dt.float32)
    basex_d = din("basex", [9, NP], dt.float32)
    rmask_d = din("rmask", [128, 2 * NROW], dt.bfloat16)
    y_d = nc.dram_tensor("y", [128, 2 * NP], dt.float32, kind="ExternalOutput")
    DBG = bool(__import__("os").environ.get("KDEBUG"))
    if DBG:
        dbg_out1_d = nc.dram_tensor("dbg_out1", [128, NPIX1 * 2], dt.bfloat16, kind="ExternalOutput")
        dbg_iA_d = nc.dram_tensor("dbg_iA", [9, NP * 2], dt.int16, kind="ExternalOutput")
        dbg_gA_d = nc.dram_tensor("dbg_gA", [9, NP * 2], dt.bfloat16, kind="ExternalOutput")
        dbg_gB_d = nc.dram_tensor("dbg_gB", [9, NP * 2], dt.bfloat16, kind="ExternalOutput")
        dbg_om_d = nc.dram_tensor("dbg_om", [128, 2048], dt.float32, kind="ExternalOutput")
        dbg_rhs_d = nc.dram_tensor("dbg_rhs", [128, NP * 2], dt.bfloat16, kind="ExternalOutput")

    with tile.TileContext(nc) as tc:
        with (
            tc.tile_pool(name="wts", bufs=1) as wp,
            tc.tile_pool(name="data", bufs=1) as dp,
            tc.tile_pool(name="pp", bufs=1) as pp,
            tc.tile_pool(name="dram", bufs=1, space="DRAM") as drp,
        ):
            w1_t = wp.tile([128, 2, 9, 2, 128], dt.bfloat16)
            om_t = wp.tile([128, 2, 9, 73], dt.bfloat16)
            dw_t = wp.tile([128, 2, 9, 2, 128], dt.bfloat16)
            dn_t = wp.tile([128, 2, 2, 128], dt.bfloat16)
            bnv_t = wp.tile([128, 14], dt.float32)
            v27_t = wp.tile([96, 4], dt.float32)
            rmask_t = wp.tile([128, 2 * NROW], dt.bfloat16)
            x_t = wp.tile([128, 2, NXR, WPAD], dt.bfloat16)
            nc.sync.dma_start(x_t[:], x_pad_d[:].rearrange(
                "p (c r w) -> p c r w", c=2, r=NXR))
            nc.sync.dma_start(w1_t[:], w1_d[:].rearrange(
                "p (c t o q) -> p c t o q", c=2, t=9, o=2))
            nc.sync.dma_start(om_t[:], om_d[:].rearrange(
                "p (c t j) -> p c t j", c=2, t=9))
            nc.sync.dma_start(dw_t[:], dw_d[:].rearrange(
                "p (c t o q) -> p c t o q", c=2, t=9, o=2))
            nc.sync.dma_start(dn_t[:], dn_d[:].rearrange(
                "p (c o q) -> p c o q", c=2, o=2))
            nc.sync.dma_start(bnv_t[:], bnv_d[:])
            nc.sync.dma_start(v27_t[:], vec27_d[:])
            nc.sync.dma_start(rmask_t[:], rmask_d[:])

            out1 = dp.tile([128, NPIX1, 2], dt.bfloat16)
            res_t = dp.tile([128, 2, NP], dt.float32)
            nc.vector.memset(out1[:], 0.0)

            out1v = out1[:].rearrange("p (r w) c -> p r w c", r=NROW)

            # ---- conv1 -> out1 (40 rows, 5 blocks of 8)
            with tc.tile_pool(name="psA", bufs=2, space="PSUM") as psA:
                for blk in range(5):
                    for ohi in range(2):
                        ps = psA.tile([128, 512], dt.float32, tag="c1")
                        for chi in range(2):
                            for t in range(9):
                                ky, kx = t // 3, t % 3
                                roff = (blk * 8 + ky) * WPAD + kx
                                nc.tensor.matmul(
                                    ps[:],
                                    w1_t[:, chi, t, ohi, :],
                                    x_t[:, chi, blk * 8 + ky:blk * 8 + ky + 8,
                                        kx:kx + 64],
                                    start=(chi == 0 and t == 0),
                                    stop=(chi == 1 and t == 8),
                                )
                        nc.scalar.activation(
                            out1v[:, blk * 8:blk * 8 + 8, 1:65, ohi],
                            ps[:].rearrange("p (r w) -> p r w", r=8),
                            Act.Relu,
                            bias=bnv_t[:, 1 + 2 * ohi:2 + 2 * ohi],
                            scale=bnv_t[:, 0 + 2 * ohi:1 + 2 * ohi],
                        )
            nc.vector.tensor_tensor(
                out1v, out1v,
                rmask_t[:].rearrange("p (c r) -> p r c", c=2).unsqueeze(2).to_broadcast(
                    [128, NROW, WPAD, 2]),
                Alu.mult)


            with tc.tile_pool(name="psB", bufs=1, space="PSUM") as psB:
                om_ps = psB.tile([128, 4, 512], dt.float32, tag="om")
                # ---- offset/mod conv (32 out rows, 4 quarters)
                for q in range(4):
                    for chi in range(2):
                        for t in range(9):
                            ky, kx = t // 3, t % 3
                            nc.tensor.matmul(
                                om_ps[0:73, q, :],
                                om_t[:, chi, t, :],
                                out1v[:, 2 + q * 8 + ky:10 + q * 8 + ky,
                                      kx:kx + 64, chi],
                                start=(chi == 0 and t == 0),
                                stop=(chi == 1 and t == 8),
                            )

                # ---- downsample 1x1 + bn3 -> res
                with tc.tile_pool(name="psC", bufs=2, space="PSUM") as psC:
                    for ohi in range(2):
                        for q in range(4):
                            ps = psC.tile([128, 512], dt.float32, tag="dn")
                            for chi in range(2):
                                nc.tensor.matmul(
                                    ps[:], dn_t[:, chi, ohi, :],
                                    x_t[:, chi, 4 + q * 8:12 + q * 8, 1:65],
                                    start=(chi == 0), stop=(chi == 1),
                                )
                            nc.scalar.activation(
                                res_t[:, ohi, q * 512:(q + 1) * 512], ps[:], Act.Copy,
                                scale=bnv_t[:, 8 + 2 * ohi:9 + 2 * ohi])

                # ---- per-pixel planes [9, NP] fp32
                omf = om_ps[:].rearrange("p q n -> p (q n)")
                if DBG:
                    om_sb = pp.tile([128, 2048], dt.float32, tag="omdbg")
                    nc.vector.memset(om_sb[:], 0.0)
                    nc.vector.tensor_copy(om_sb[0:73, :], omf[0:73, :])
                    nc.sync.dma_start(dbg_om_d[:], om_sb[:])
                gamA = pp.tile([9, NP, 2], dt.bfloat16, tag="gamA")
                gamB = pp.tile([9, NP, 2], dt.bfloat16, tag="gamB")
                iA = pp.tile([9, NP, 2], dt.int16, tag="iA")
                tmp_pool = tc.tile_pool(name="pptmp", bufs=1)
                tp = tmp_pool.__enter__()
                basey_t = tp.tile([9, NP], dt.float32, tag="by")
                basex_t = tp.tile([9, NP], dt.float32, tag="bx")
                nc.sync.dma_start(basey_t[:], basey_d[:])
                nc.sync.dma_start(basex_t[:], basex_d[:])
                t0 = tp.tile([9, NP], dt.float32, tag="t0")
                t1 = tp.tile([9, NP], dt.float32, tag="t1")
                t2 = tp.tile([9, NP], dt.float32, tag="t2")
                t3 = tp.tile([9, NP], dt.float32, tag="t3")
                t4 = tp.tile([9, NP], dt.float32, tag="t4")
                t5 = tp.tile([9, NP], dt.float32, tag="t5")

                nc.vector.scalar_tensor_tensor(
                    t0[:], omf[0:9, :], v27_t[0:9, 0:1], basey_t[:], Alu.add, Alu.add)
                nc.vector.tensor_scalar(t0[:], t0[:], -1.5, 64.5, Alu.max, Alu.min)
                nc.vector.tensor_scalar(t1[:], t0[:], MAGIC, None, Alu.add)
                nc.vector.tensor_scalar(t1[:], t1[:], -MAGIC, None, Alu.add)
                nc.vector.scalar_tensor_tensor(
                    t2[:], t0[:], 0.5, t1[:], Alu.add, Alu.subtract)   # fy
                nc.vector.scalar_tensor_tensor(
                    t3[:], omf[32:41, :], v27_t[0:9, 2:3], basex_t[:], Alu.add, Alu.add)
                nc.vector.tensor_scalar(t3[:], t3[:], -1.5, 64.5, Alu.max, Alu.min)
                nc.vector.tensor_scalar(t4[:], t3[:], MAGIC, None, Alu.add)
                nc.vector.tensor_scalar(t4[:], t4[:], -MAGIC, None, Alu.add)
                nc.vector.scalar_tensor_tensor(
                    t5[:], t3[:], 0.5, t4[:], Alu.add, Alu.subtract)   # fx
                # flat00 = y0f*66 + rowc + x0f -> t1
                nc.vector.tensor_scalar(
                    t1[:], t1[:], 66.0, v27_t[0:9, 1:2], Alu.mult, Alu.add)
                nc.vector.tensor_tensor(t1[:], t1[:], t4[:], Alu.add)
                nc.vector.tensor_copy(iA[:, :, 0], t1[:])
                nc.vector.tensor_scalar(iA[:, :, 1], t1[:], 1.0, None, Alu.add)
                # m2 = sigmoid(mod+mod_b) -> t0 (reuse)
                nc.scalar.activation(t0[:], omf[64:73, :], Act.Sigmoid,
                                     bias=v27_t[0:9, 3:4])
                nc.vector.tensor_scalar(t4[:], t2[:], -2.0, 2.0, Alu.mult, Alu.add)
                nc.vector.tensor_tensor(t1[:], t4[:], t0[:], Alu.mult)   # mwy0
                nc.vector.tensor_scalar(t4[:], t2[:], 2.0, None, Alu.mult)
                nc.vector.tensor_tensor(t3[:], t4[:], t0[:], Alu.mult)   # mwy1
                nc.vector.tensor_scalar(t4[:], t5[:], -1.0, 1.0, Alu.mult, Alu.add)
                nc.vector.tensor_tensor(gamA[:, :, 0], t1[:], t4[:], Alu.mult)
                nc.vector.tensor_tensor(gamA[:, :, 1], t1[:], t5[:], Alu.mult)
                nc.vector.tensor_tensor(gamB[:, :, 0], t3[:], t4[:], Alu.mult)
                nc.vector.tensor_tensor(gamB[:, :, 1], t3[:], t5[:], Alu.mult)

                tmp_pool.__exit__(None, None, None)

            if DBG:
                nc.sync.dma_start(dbg_out1_d[:], out1[:].rearrange("p n c -> p (n c)"))
                nc.sync.dma_start(dbg_iA_d[:], iA[:].rearrange("p n c -> p (n c)"))
                nc.sync.dma_start(dbg_gA_d[:], gamA[:].rearrange("p n c -> p (n c)"))
                nc.sync.dma_start(dbg_gB_d[:], gamB[:].rearrange("p n c -> p (n c)"))

            # idx -> DRAM scratch for wrap reload
            idram = drp.tile([9, NP * 2], dt.int16)
            nc.sync.dma_start(idram[:], iA[:].rearrange("p n c -> p (n c)"))

            # ---- gather + lerp + deform matmuls
            with tc.tile_pool(name="gat", bufs=2) as gp, \
                 tc.tile_pool(name="psD", bufs=1, space="PSUM") as psD:
                def_ps = psD.tile([128, 2, 4, 512], dt.float32, tag="def")
                for k in range(9):
                    iw = gp.tile([128, 256], dt.int16, tag="iw")
                    nc.sync.dma_start(
                        iw[0:16, :],
                        idram[k:k + 1, :].rearrange("o (s p) -> (o p) s", p=16))
                    kk = 16
                    while kk < 128:
                        nc.sync.dma_start(iw[kk:2 * kk, :], iw[0:kk, :])
                        kk *= 2
                    gbcA = gp.tile([128, NP * 2], dt.bfloat16, tag="gbcA")
                    gbcB = gp.tile([128, NP * 2], dt.bfloat16, tag="gbcB")
                    for (gbct, nat) in ((gbcA, gamA), (gbcB, gamB)):
                        nc.sync.dma_start(
                            gbct[0:1, :],
                            nat[k:k + 1, :, :].rearrange("o n c -> o (n c)"))
                        kk = 1
                        while kk < 128:
                            nc.sync.dma_start(gbct[kk:2 * kk, :], gbct[0:kk, :])
                            kk *= 2
                    rhs = gp.tile([128, NP, 2], dt.bfloat16, tag="rhs")
                    for pas in range(2):
                        g = gp.tile([128, 2 * NP, 2], dt.bfloat16, tag="g")
                        if pas == 0:
                            nc.gpsimd.ap_gather(
                                g[:], out1[:, 0:NPIX1, :], iw[:],
                                channels=128, num_elems=NPIX1, d=2, num_idxs=2 * NP)
                        else:
                            nc.gpsimd.ap_gather(
                                g[:], out1[:, 66:NPIX1, :], iw[:],
                                channels=128, num_elems=NPIX1 - 66, d=2,
                                num_idxs=2 * NP)
                        gbct = gbcA if pas == 0 else gbcB
                        nc.vector.tensor_tensor(
                            g[:], g[:],
                            gbct[:].unsqueeze(2).to_broadcast([128, 2 * NP, 2]),
                            Alu.mult)
                        gv = g[:].rearrange("p (u n) c -> p u n c", n=2)
                        if pas == 0:
                            nc.vector.tensor_tensor(
                                rhs[:], gv[:, :, 0, :], gv[:, :, 1, :], Alu.add)
                        else:
                            nc.vector.tensor_tensor(
                                rhs[:], rhs[:], gv[:, :, 0, :], Alu.add)
                            nc.vector.tensor_tensor(
                                rhs[:], rhs[:], gv[:, :, 1, :], Alu.add)
                    if DBG and k == 4:
                        nc.sync.dma_start(dbg_rhs_d[:], rhs[:].rearrange("p n c -> p (n c)"))
                    for ohi in range(2):
                        for ut in range(4):
                            for chi in range(2):
                                nc.tensor.matmul(
                                    def_ps[:, ohi, ut, :],
                                    dw_t[:, chi, k, ohi, :],
                                    rhs[:, ut * 512:(ut + 1) * 512, chi],
                                    start=(k == 0 and chi == 0),
                                    stop=(k == 8 and chi == 1),
                                )

                # ---- final: y = relu(def*s2 + (t2+t3) + res)
                for ohi in range(2):
                    nc.vector.scalar_tensor_tensor(
                        res_t[:, ohi, :],
                        def_ps[:, ohi, :, :].rearrange("p a b -> p (a b)"),
                        bnv_t[:, 4 + 2 * ohi:5 + 2 * ohi],
                        res_t[:, ohi, :], Alu.mult, Alu.add)
                    nc.scalar.activation(
                        res_t[:, ohi, :], res_t[:, ohi, :], Act.Relu,
                        bias=bnv_t[:, 5 + 2 * ohi:6 + 2 * ohi])
            nc.sync.dma_start(y_d[:], res_t[:].rearrange("p a b -> p (a b)"))

    nc.compile()
    return nc


def _get_nc():
    global _NC
    if _NC is None:
        _NC = _build()
    return _NC


def _prep_inputs(inputs):
    f32 = np.float32
    x = np.asarray(inputs['x'], f32)
    conv1_w = np.asarray(inputs['conv1_w'], f32)
    off_w = np.asarray(inputs['off_w'], f32)
    mod_w = np.asarray(inputs['mod_w'], f32)
    def_w = np.asarray(inputs['def_w'], f32)
    down_w = np.asarray(inputs['down_w'], f32)

    def wT(w):  # [O,C,3,3] -> [c_lo][c_hi,tap,o_hi,o_lo]
        O, C = w.shape[0], w.shape[1]
        t = w.reshape(O, C, 9).transpose(1, 2, 0)            # [C,9,O]
        t = t.reshape(2, 128, 9, 2, 128).transpose(1, 0, 2, 3, 4)
        return np.ascontiguousarray(t).astype(BF16).reshape(128, -1)

    w1 = wT(conv1_w)
    dw = wT(def_w)
    omw = np.zeros((73, 256, 9), f32)
    omw[0:9] = off_w.reshape(18, 256, 9)[0::2]
    omw[32:41] = off_w.reshape(18, 256, 9)[1::2]
    omw[64:73] = mod_w.reshape(9, 256, 9)
    om = omw.transpose(1, 2, 0).reshape(2, 128, 9, 73).transpose(1, 0, 2, 3)
    om = np.ascontiguousarray(om).astype(BF16).reshape(128, -1)
    dn = down_w.reshape(256, 256).T.reshape(2, 128, 2, 128).transpose(1, 0, 2, 3)
    dn = np.ascontiguousarray(dn).astype(BF16).reshape(128, -1)

    eps = 1e-5

    def fold(g, b, m, v, cb=None):
        s = np.asarray(g, f32) / np.sqrt(np.asarray(v, f32) + eps)
        t = np.asarray(b, f32) - np.asarray(m, f32) * s
        if cb is not None:
            t = t + np.asarray(cb, f32) * s
        return s, t

    s1, t1 = fold(inputs['bn1_g'], inputs['bn1_b'], inputs['bn1_m'],
                  inputs['bn1_v'], inputs['conv1_b'])
    s2, t2 = fold(inputs['bn2_g'], inputs['bn2_b'], inputs['bn2_m'], inputs['bn2_v'])
    s3, t3 = fold(inputs['bn3_g'], inputs['bn3_b'], inputs['bn3_m'],
                  inputs['bn3_v'], inputs['down_b'])
    bnv = np.zeros((128, 14), f32)
    for i, vv in enumerate([s1, t1, s2, t2 + t3, s3, t3 * 0.0]):
        v2 = vv.reshape(2, 128)
        base = (i // 2) * 4 + (i % 2)
        bnv[:, base] = v2[0]
        bnv[:, base + 2] = v2[1]

    off_b = np.asarray(inputs['off_b'], f32)
    mod_b = np.asarray(inputs['mod_b'], f32)
    v27 = np.zeros((96, 4), f32)
    v27[0:9, 0] = off_b[0::2]
    v27[0:9, 2] = off_b[1::2]
    v27[0:9, 3] = mod_b

    ky = np.repeat(np.arange(3, dtype=f32), 3)
    kx = np.tile(np.arange(3, dtype=f32), 3)
    prow = np.repeat(np.arange(32, dtype=f32), 64)
    pcol = np.tile(np.arange(64, dtype=f32), 32)

    in_maps = []
    for core in range(8):
        b, h = core // 2, core % 2
        xp = np.zeros((2, 128, NXR, WPAD), f32)
        lo = 32 * h - 4
        xr = x[b].reshape(2, 128, 64, 64)
        for i in range(NXR):
            r = lo + i
            if 0 <= r < 64:
                xp[:, :, i, 1:65] = xr[:, :, r, :]
        xp = np.ascontiguousarray(xp.transpose(1, 0, 2, 3)).astype(BF16)

        basey = (prow[None, :] + 32 * h) + ky[:, None] - 1.5
        basex = pcol[None, :] + kx[:, None] - 1.5
        v27c = v27.copy()
        v27c[0:9, 1] = 1.0 - 66.0 * (32 * h - 3)
        rmask = np.zeros((2, NROW), f32)
        for i in range(NROW):
            r = 32 * h - 3 + i
            rmask[:, i] = 1.0 if 0 <= r < 64 else 0.0
        rm = np.ascontiguousarray(
            np.broadcast_to(rmask.reshape(1, 2 * NROW), (128, 2 * NROW))).astype(BF16)
        in_maps.append(dict(
            x_pad=xp.reshape(128, -1), w1=w1, om=om, dw=dw, dn=dn, bnv=bnv,
            vec27=v27c, basey=np.ascontiguousarray(basey, f32),
            basex=np.ascontiguousarray(basex, f32), rmask=rm,
        ))
    return in_maps


def kernel(**inputs):
    from concourse import bass_utils
    nc = _get_nc()
    in_maps = _prep_inputs(inputs)
    res = bass_utils.run_bass_kernel_spmd(nc, in_maps, list(range(8)), trace=False)
    out = np.zeros((B, COUT, H, W), np.float32)
    for core in range(8):
        b, h = core // 2, core % 2
        y = res.results[core]["y"].reshape(128, 2, 32, 64)
        out[b, :, 32 * h:32 * h + 32, :] = y.transpose(1, 0, 2, 3).reshape(256, 32, 64)
    return out
